# revision 16
# baseline (speedup 1.0000x reference)
"""Trainium2 Bass kernel for BatchedActivationCSA.

Math: the reference computes, per token vector x (1024-dim):
    z   = FWHT(permute(x * signs))[:64]          (linear -> 64x1024 matrix T)
    g   = gate * z                               (per-batch gate)
    sp  = keep g_i iff |g_i| in top-16 of |g| AND |g_i| >= tau
    r   = permute^-1(FWHT(pad_64->1024(alpha*sp))) * signs   (linear -> T^T)
    out = x + r
Both linear maps are the SAME 64x1024 matrix A (FWHT is symmetric/orthonormal,
verified numerically), so the device kernel is just:
    G   = X @ A1^T        with A1 = diag(gate) @ A      [per-batch, host-built]
    SP  = topk16/tau threshold of G  (Max8 + MatchReplace + Max8 -> 16th max)
    OUT = X + SP @ A2     with A2 = alpha * A           [per-batch, host-built]
Top-16 selection == |g| >= (16th largest of |g|), exact for tie-free data.

Sharding: 8 cores, core c handles batch b=c//2, seq half c%2 -> 2048 tokens.
A1/A2/tau differ per core (per batch); same SPMD program on all cores.
"""

import numpy as np

BSZ, SEQ, DIM = 4, 4096, 1024
M = 64            # measure dim
NCORES = 8
TOK = BSZ * SEQ // NCORES      # 2048 tokens per core
TPT = 256                      # tokens per macro tile (128 partitions x 2)
NT = TOK // TPT                # 8 macro tiles per core

_cache = {}


def _fwht(y):
    """Walsh-Hadamard over last dim, identical ordering to the reference."""
    n = y.shape[-1]
    lead = y.shape[:-1]
    out = y.copy()
    h = 1
    while h < n:
        out = out.reshape(*lead, -1, 2, h)
        a, b = out[..., 0, :], out[..., 1, :]
        out = np.concatenate((a + b, a - b), axis=-1).reshape(*lead, n)
        h *= 2
    return out * (n ** -0.5)


def _build_nc():
    import concourse.bass as bass
    import concourse.mybir as mybir
    from concourse.tile import TileContext
    from concourse.masks import make_identity

    f32 = mybir.dt.float32
    f16 = mybir.dt.float16
    ACT = mybir.ActivationFunctionType
    ALU = mybir.AluOpType

    nc = bass.Bass()

    x_d = nc.dram_tensor("x", [TOK, DIM], f32, kind="ExternalInput")
    a1t_d = nc.dram_tensor("a1t", [128, 8 * M], f32, kind="ExternalInput")
    a2_d = nc.dram_tensor("a2", [M, DIM], f16, kind="ExternalInput")
    tau_d = nc.dram_tensor("tau", [128, 1], f32, kind="ExternalInput")
    out_d = nc.dram_tensor("out", [TOK, DIM], f32, kind="ExternalOutput")

    # [2048, 1024] -> [8 tiles, 128 partitions, 2*1024]; partition p of tile t
    # holds tokens t*256+2p (cols 0:1024) and t*256+2p+1 (cols 1024:2048).
    xv = x_d[:, :].rearrange("(t p two) d -> t p (two d)", p=128, two=2)
    ov = out_d[:, :].rearrange("(t p two) d -> t p (two d)", p=128, two=2)

    with TileContext(nc) as tc:
        with (
            tc.tile_pool(name="const", bufs=1) as consts,
            tc.tile_pool(name="xin", bufs=3) as xin_pool,
            tc.tile_pool(name="xt", bufs=3) as xt_pool,
            tc.tile_pool(name="oout", bufs=3) as out_pool,
            tc.tile_pool(name="small", bufs=4) as small,
            tc.tile_pool(name="ps_t", bufs=2, space="PSUM") as ps_t,
            tc.tile_pool(name="ps_g", bufs=2, space="PSUM") as ps_g,
            tc.tile_pool(name="ps_s", bufs=2, space="PSUM") as ps_s,
            tc.tile_pool(name="ps_o", bufs=2, space="PSUM") as ps_o,
        ):
            a1t_s = consts.tile([128, 8 * M], f32)
            nc.sync.dma_start(a1t_s, a1t_d[:, :])
            a2_s = consts.tile([M, DIM], f16)
            nc.sync.dma_start(a2_s, a2_d[:, :])
            tau_s = consts.tile([128, 1], f32)
            nc.sync.dma_start(tau_s, tau_d[:, :])
            ident = consts.tile([128, 128], f32)
            make_identity(nc, ident)
            ident16 = consts.tile([128, 128], f16)
            make_identity(nc, ident16)

            def emit_sense(t, x_s):
                """transposes + mm1 + shrink chain for tile t; returns the
                per-group sparse fp16 tiles."""
                sps = []
                for g in range(2):  # token subgroup: even / odd tokens
                    gofs = g * DIM
                    xt_s = xt_pool.tile([128, DIM], f32, tag="xt")
                    for half in range(2):
                        pt = ps_t.tile([128, 512], f32, tag="pt")
                        for cc in range(4):
                            c = half * 4 + cc
                            nc.tensor.transpose(
                                pt[:, cc * 128:(cc + 1) * 128],
                                x_s[:, gofs + c * 128: gofs + (c + 1) * 128],
                                ident,
                            )
                        nc.scalar.activation(
                            xt_s[:, half * 512:(half + 1) * 512], pt, ACT.Copy
                        )
                    gp = ps_g.tile([128, M], f32, tag="g")
                    for c in range(8):
                        nc.tensor.matmul(
                            gp,
                            lhsT=xt_s[:, c * 128:(c + 1) * 128],
                            rhs=a1t_s[:, c * M:(c + 1) * M],
                            start=(c == 0),
                            stop=(c == 7),
                        )
                    ag = small.tile([128, M], f32, tag="ag")
                    nc.scalar.activation(ag, gp, ACT.Abs)
                    g_sb = small.tile([128, M], f32, tag="gsb")
                    nc.scalar.activation(g_sb, gp, ACT.Copy)
                    m8a = small.tile([128, 8], f32, tag="m8a")
                    nc.vector.max(m8a, ag)
                    agr = small.tile([128, M], f32, tag="agr")
                    nc.vector.match_replace(agr, m8a, ag, -1.0)
                    m8b = small.tile([128, 8], f32, tag="m8b")
                    nc.vector.max(m8b, agr)
                    thr = small.tile([128, 1], f32, tag="thr")
                    nc.gpsimd.tensor_single_scalar(
                        thr, m8b[:, 7:8], tau_s[:, 0:1], ALU.max
                    )
                    mask = small.tile([128, M], f32, tag="mask")
                    nc.vector.tensor_single_scalar(
                        mask, ag, thr[:, 0:1], ALU.is_ge
                    )
                    sp = small.tile([128, M], f16, tag="sp")
                    nc.vector.tensor_tensor(sp, mask, g_sb, ALU.mult)
                    sps.append(sp)
                return sps

            def emit_recon(t, x_s, o_s, sps):
                """sparse-transpose + mm2 + add + store for tile t."""
                for g in range(2):
                    gofs = g * DIM
                    stp = ps_s.tile([M, 128], f16, tag="st")
                    nc.tensor.transpose(stp, sps[g], ident16)
                    st_s = small.tile([M, 128], f16, tag="sts")
                    nc.scalar.activation(st_s, stp, ACT.Copy)
                    for h in range(2):
                        op = ps_o.tile([128, 512], f32, tag="op")
                        nc.tensor.matmul(
                            op,
                            lhsT=st_s,
                            rhs=a2_s[:, h * 512:(h + 1) * 512],
                            start=True,
                            stop=True,
                        )
                        nc.vector.tensor_tensor(
                            o_s[:, gofs + h * 512: gofs + (h + 1) * 512],
                            op,
                            x_s[:, gofs + h * 512: gofs + (h + 1) * 512],
                            ALU.add,
                        )
                nc.scalar.dma_start(ov[t], o_s)

            # software pipeline: sense(t) overlaps recon(t-1) so the PE never
            # waits on the cross-engine shrink chain (keeps HAM clock warm)
            prev = None
            for t in range(NT):
                x_s = xin_pool.tile([128, 2 * DIM], f32, tag="x")
                nc.sync.dma_start(x_s, xv[t])
                o_s = out_pool.tile([128, 2 * DIM], f32, tag="o")
                sps = emit_sense(t, x_s)
                if prev is not None:
                    emit_recon(*prev)
                prev = (t, x_s, o_s, sps)
            emit_recon(*prev)

    _split_pe_waits(nc, mybir)
    return nc


def _split_pe_waits(nc, mybir):
    """walrus codegen allows only one sync wait on most compute instruction
    structs (PE LDWEIGHTS, DVE TS, ...). Move the waits of any multi-wait
    compute instruction onto a NoOp inserted just before it: each engine's
    sequencer executes in order, so all waits still happen-before it."""
    skip = (
        mybir.InstNoOp,
        mybir.InstEventSemaphore,
        mybir.InstUnconditionalBranch,
        mybir.InstRegisterMove,
    )
    for f in nc.m.functions:
        for blk in f.blocks:
            insts = list(blk.instructions)
            out = []
            changed = False
            for ins in insts:
                si = getattr(ins, "sync_info", None)
                if (
                    not isinstance(ins, skip)
                    and getattr(ins, "engine", None) is not None
                    and si is not None
                    and si.on_wait
                    and len(si.on_wait) > 1
                ):
                    waits = list(si.on_wait)
                    for k, w in enumerate(waits[:-1]):
                        nop = mybir.InstNoOp(
                            name=f"{ins.name}-waitsplit{k}", ins=[], outs=[]
                        )
                        nop.engine = ins.engine
                        nop.sync_info = mybir.SyncInfo(
                            on_wait=[w], on_update=[]
                        )
                        out.append(nop)
                    ins.sync_info = mybir.SyncInfo(
                        on_wait=[waits[-1]], on_update=list(si.on_update)
                    )
                    changed = True
                out.append(ins)
            if changed:
                blk.instructions = out


def _prep_inputs(x, gates, alpha, tau, signs, perm, inv_perm, target_idx):
    """Host-side prep: build per-core input maps (small matrices only)."""
    tidx = int(target_idx)
    signs = np.asarray(signs, dtype=np.float64)
    perm = np.asarray(perm, dtype=np.int64)
    inv_perm = np.asarray(inv_perm, dtype=np.int64)

    # Sense matrix A: row i = i-th output of FWHT(permute(e * signs))[:64].
    eye = np.eye(DIM, dtype=np.float64)
    A = _fwht((eye * signs[None, :])[:, perm])[:, :M].T          # [64, 1024]
    # Reconstruct matrix B (provably == A, but built independently for safety)
    pad = np.zeros((M, DIM), dtype=np.float64)
    pad[:, :M] = np.eye(M)
    B = _fwht(pad)[:, inv_perm] * signs[None, :]                 # [64, 1024]

    in_maps = []
    for c in range(NCORES):
        b, half = divmod(c, 2)
        g = np.asarray(gates, dtype=np.float64)[b, tidx]         # [64]
        al = float(np.asarray(alpha, dtype=np.float64)[b, tidx, 0])
        tu = abs(float(np.asarray(tau, dtype=np.float64)[b, tidx, 0]))
        A1 = (g[:, None] * A).astype(np.float32)                 # [64, 1024]
        A2 = (al * B).astype(np.float32)                         # [64, 1024]
        # a1t layout: [128, 8*64]; cols c*64:(c+1)*64 = A1.T[c*128:(c+1)*128]
        a1t = np.ascontiguousarray(
            A1.T.reshape(8, 128, M).transpose(1, 0, 2).reshape(128, 8 * M)
        )
        xs = np.ascontiguousarray(
            np.asarray(x)[b, half * TOK:(half + 1) * TOK, :], dtype=np.float32
        )
        in_maps.append({
            "x": xs,
            "a1t": a1t,
            "a2": np.ascontiguousarray(A2.astype(np.float16)),
            "tau": np.full((128, 1), tu, dtype=np.float32),
        })
    return in_maps


def _get_nc():
    if "nc" not in _cache:
        _cache["nc"] = _build_nc()
    return _cache["nc"]


def kernel(x, gates, alpha, tau, signs, perm, inv_perm, target_idx,
           _trace=False, _tmpdir=None):
    from concourse.bass_utils import run_bass_kernel_spmd

    nc = _get_nc()
    in_maps = _prep_inputs(x, gates, alpha, tau, signs, perm, inv_perm,
                           target_idx)
    res = run_bass_kernel_spmd(
        nc, in_maps, core_ids=list(range(NCORES)),
        trace=_trace, tmpdir=_tmpdir,
    )
    if _trace:
        _cache["last_results"] = res
    out = np.empty((BSZ, SEQ, DIM), dtype=np.float32)
    for c in range(NCORES):
        b, half = divmod(c, 2)
        out[b, half * TOK:(half + 1) * TOK, :] = res.results[c]["out"]
    return out
